# revision 5
# baseline (speedup 1.0000x reference)
"""Masked scaled-dot-product attention (B=8,H=8,S=1024,D=64) on 8 trn2 cores.

Sharding: batch across the 8 cores (attention is independent per batch/head).
Per core: one batch -> q,k,v [H,S,D], mask [S,S] i32, adj [S-1,S-1] f32.
Outputs per core: res [H,S,D], p_attn [H,S,S].

Per (head, 128-row q-tile) pipeline:
  PE  : S = (0.125*Q)^T.T @ K^T       (Q,K pre-transposed on PE via identity)
  DVE : masked = S + M_add            (M_add = 0 / -1e9, precomputed per batch)
  ACT : P = exp(masked), rowsum accumulated in the same instruction
  DVE : recip = 1/rowsum ; Pn = P * recip  -> DMA out p_attn
  PE  : transpose P (8x 128x128)      -> PSUM
  DVE : copy P^T PSUM->SBUF
  PE  : out = P^T.T @ V (accumulate over 8 k-chunks)
  DVE : res = out * recip             -> DMA out res
No max-subtraction: scores/8 ~ N(0,1) so exp cannot overflow; masked entries
get -1e9 and underflow to exactly 0.0 the same way the reference does.
"""

import numpy as np

import concourse.bass as bass
from concourse import bacc
import concourse.mybir as mybir
from concourse.tile import TileContext
from concourse.masks import make_identity
from concourse.bass_utils import run_bass_kernel_spmd

B, H, S, D = 8, 8, 1024, 64
P = 128
NQ = S // P  # 8 row tiles
NEG = -1.0e9
F32 = mybir.dt.float32
I32 = mybir.dt.int32

_cache = {}


def _build_nc():
    nc = bacc.Bacc("TRN2", target_bir_lowering=False, debug=True)
    q = nc.dram_tensor("query", [H, S, D], F32, kind="ExternalInput")
    k = nc.dram_tensor("key", [H, S, D], F32, kind="ExternalInput")
    v = nc.dram_tensor("value", [H, S, D], F32, kind="ExternalInput")
    mask = nc.dram_tensor("mask", [S, S], I32, kind="ExternalInput")
    adj = nc.dram_tensor("adj", [S - 1, S - 1], F32, kind="ExternalInput")
    res = nc.dram_tensor("res", [H, S, D], F32, kind="ExternalOutput")
    p_out = nc.dram_tensor("p_attn", [H, S, S], F32, kind="ExternalOutput")

    Exp = mybir.ActivationFunctionType.Exp
    Copy = mybir.ActivationFunctionType.Copy
    gt = mybir.AluOpType.is_gt

    with TileContext(nc) as tc:
        with (
            tc.tile_pool(name="consts", bufs=1) as consts,
            tc.tile_pool(name="madd", bufs=1) as madd_pool,
            tc.tile_pool(name="setup", bufs=2) as setup,
            tc.tile_pool(name="head", bufs=2) as head_pool,
            tc.tile_pool(name="work", bufs=2) as work,
            tc.tile_pool(name="stats", bufs=3) as stats,
            tc.tile_pool(name="psum_s", bufs=2, space="PSUM") as psum_s,
            tc.tile_pool(name="psum_pt", bufs=1, space="PSUM") as psum_pt,
            tc.tile_pool(name="psum_sm", bufs=1, space="PSUM") as psum_sm,
        ):
            identity = consts.tile([P, P], F32)
            make_identity(nc, identity)

            # ---- per-batch additive mask: M_add = combined ? 0 : -1e9 ----
            m_tiles = []
            for qi in range(NQ):
                mask_t = setup.tile([P, S], I32, tag="mask_ld")
                nc.sync.dma_start(mask_t, mask[qi * P:(qi + 1) * P, :])
                adj_t = setup.tile([P, S], F32, tag="adj_ld")
                nc.vector.memset(adj_t, 1.0)
                if qi < NQ - 1:
                    nc.sync.dma_start(adj_t[:, 0:S - 1], adj[qi * P:(qi + 1) * P, :])
                else:
                    nc.sync.dma_start(adj_t[0:P - 1, 0:S - 1], adj[qi * P:S - 1, :])

                mask_f = setup.tile([P, S], F32, tag="mask_f")
                nc.vector.tensor_copy(mask_f, mask_t)  # i32 -> f32 cast
                comb = setup.tile([P, S], F32, tag="comb")
                # comb = (adj_p > 0) * mask_f  -> 1.0 where allowed
                nc.vector.tensor_scalar(comb, adj_t, 0.0, None, gt)
                nc.vector.tensor_mul(comb, comb, mask_f)
                m_t = madd_pool.tile([P, S], F32, tag=f"madd{qi}")
                # M_add = comb*1e9 - 1e9  (0 where allowed, -1e9 where masked)
                nc.scalar.activation(m_t, comb, Copy, bias=-1.0e9, scale=1.0e9)
                m_tiles.append(m_t)

            for h in range(H):
                # ---- per-head setup: QT (scaled), KT, V in SBUF ----
                qt = head_pool.tile([D, S], F32, tag="qt")
                kt = head_pool.tile([D, S], F32, tag="kt")
                vt = head_pool.tile([P, NQ, D], F32, tag="vt")
                for c in range(NQ):
                    sl = slice(c * P, (c + 1) * P)
                    qc = setup.tile([P, D], F32, tag="q_ld")
                    nc.sync.dma_start(qc, q[h, sl, :])
                    tr1 = psum_sm.tile([D, P], F32, tag="tr")
                    nc.tensor.transpose(tr1, qc, identity)
                    nc.vector.tensor_scalar_mul(qt[:, sl], tr1, 0.125)
                    kc = setup.tile([P, D], F32, tag="k_ld")
                    nc.sync.dma_start(kc, k[h, sl, :])
                    tr2 = psum_sm.tile([D, P], F32, tag="tr")
                    nc.tensor.transpose(tr2, kc, identity)
                    nc.vector.tensor_copy(kt[:, sl], tr2)
                    nc.sync.dma_start(vt[:, c, :], v[h, sl, :])

                for qi in range(NQ):
                    qsl = slice(qi * P, (qi + 1) * P)
                    s_ps = psum_s.tile([P, S], F32, tag="s")
                    nc.tensor.matmul(s_ps[:, 0:512], qt[:, qsl], kt[:, 0:512],
                                     start=True, stop=True)
                    nc.tensor.matmul(s_ps[:, 512:1024], qt[:, qsl], kt[:, 512:1024],
                                     start=True, stop=True)
                    msk = work.tile([P, S], F32, tag="msk")
                    nc.vector.tensor_add(msk, s_ps, m_tiles[qi])
                    p_t = work.tile([P, S], F32, tag="p")
                    rs = stats.tile([P, 1], F32, tag="rs")
                    nc.scalar.activation(p_t, msk, Exp, accum_out=rs)
                    rcp = stats.tile([P, 1], F32, tag="rcp")
                    nc.vector.reciprocal(rcp, rs)
                    pn = work.tile([P, S], F32, tag="pn")
                    nc.vector.tensor_scalar_mul(pn, p_t, rcp)
                    nc.sync.dma_start(p_out[h, qsl, :], pn)

                    pt_ps = psum_pt.tile([P, S], F32, tag="pt")
                    for c in range(NQ):
                        csl = slice(c * P, (c + 1) * P)
                        nc.tensor.transpose(pt_ps[:, csl], p_t[:, csl], identity)
                    pt_sb = work.tile([P, S], F32, tag="ptsb")
                    nc.vector.tensor_copy(pt_sb, pt_ps)
                    o_ps = psum_sm.tile([P, D], F32, tag="pv")
                    for c in range(NQ):
                        csl = slice(c * P, (c + 1) * P)
                        nc.tensor.matmul(o_ps, pt_sb[:, csl], vt[:, c, :],
                                         start=(c == 0), stop=(c == NQ - 1))
                    r_sb = stats.tile([P, D], F32, tag="r")
                    nc.vector.tensor_scalar_mul(r_sb, o_ps, rcp)
                    nc.sync.dma_start(res[h, qsl, :], r_sb)
    nc.compile()
    return nc


def kernel(query, key, value, mask, adj, layer=0, **kwargs):
    if "nc" not in _cache:
        _cache["nc"] = _build_nc()
    nc = _cache["nc"]

    query = np.ascontiguousarray(np.asarray(query, dtype=np.float32))
    key = np.ascontiguousarray(np.asarray(key, dtype=np.float32))
    value = np.ascontiguousarray(np.asarray(value, dtype=np.float32))
    mask = np.ascontiguousarray(np.asarray(mask, dtype=np.int32))
    adj = np.ascontiguousarray(np.asarray(adj, dtype=np.float32))

    in_maps = [
        {
            "query": query[b],
            "key": key[b],
            "value": value[b],
            "mask": mask[b],
            "adj": adj[b],
        }
        for b in range(B)
    ]
    out = run_bass_kernel_spmd(nc, in_maps, list(range(B)))
    results = out.results
    res = np.stack([np.asarray(r["res"]) for r in results], axis=0)
    p_attn = np.stack([np.asarray(r["p_attn"]) for r in results], axis=0)
    return res, p_attn
